# revision 9
# baseline (speedup 1.0000x reference)
"""GCN baseline kernel for Trainium2, data-parallel over 8 NeuronCores.

Math (per graph, N=400 nodes, D=128 channels):
  A_hat = D^{-1/2} (sc + I) D^{-1/2}
  x0 = sc
  x_{l+1} = relu_l( A_hat @ (x_l @ W'_l) + t'_l )      (BN folded into W', t')
  h = x3.reshape(N*D);  MLP 51200->256->64->2 with LayerNorm+relu between.

Device layout choices:
  * Host ships sc TRANSPOSED per graph (sct[g, m, n] = sc[g, n, m]) in bf16.
  * Features live channel-major (x^T, [D, N]); z = x @ W is computed
    node-major ([N, D] in 128-row chunks) using x^T chunks as the stationary
    operand, so no explicit transposes are needed in the GCN layers.
  * Aggregation v^T = sum_m u[m,:] * A_hatT[m,:] accumulates in one PSUM bank.
  * Degree row-sums ride along the z1 pass as free=1 matmuls against a ones
    vector, landing node-major so dinv is directly usable as a per-partition
    scalar.
  * The MLP head is batched over all 64 graphs of the core (x3 of every graph
    is kept on SBUF channel-major), so w1 is streamed once per core.
"""

import numpy as np
import ml_dtypes

import concourse.bass as bass
import concourse.bacc as bacc
import concourse.tile as tile
from concourse import mybir
from concourse.bass_utils import run_bass_kernel_spmd
from concourse.masks import make_identity

F32 = mybir.dt.float32
BF16 = mybir.dt.bfloat16
NPBF16 = ml_dtypes.bfloat16

B, N, D = 512, 400, 128
NCORES = 8
BG = B // NCORES                    # graphs per core
NCH = (N + 127) // 128              # node chunks: 128,128,128,16
CHS = [min(128, N - 128 * c) for c in range(NCH)]
H1, H2, NCLS = 256, 64, 2
EPS_BN = 1e-5
EPS_LN = 1e-5

AF = mybir.ActivationFunctionType
OP = mybir.AluOpType


def _emit(nc, n_graphs=BG):
    sct = nc.dram_tensor("sct", [n_graphs, N, N], BF16, kind="ExternalInput")
    w0 = nc.dram_tensor("w0", [N, D], BF16, kind="ExternalInput")
    w1c = nc.dram_tensor("w1c", [D, D], BF16, kind="ExternalInput")
    w2c = nc.dram_tensor("w2c", [D, D], BF16, kind="ExternalInput")
    t0 = nc.dram_tensor("t0", [D, 1], F32, kind="ExternalInput")
    t1 = nc.dram_tensor("t1", [D, 1], F32, kind="ExternalInput")
    t2 = nc.dram_tensor("t2", [D, 1], F32, kind="ExternalInput")
    mw1 = nc.dram_tensor("mw1", [N * D, H1], BF16, kind="ExternalInput")
    b1 = nc.dram_tensor("b1", [1, H1], F32, kind="ExternalInput")
    g1 = nc.dram_tensor("g1", [1, H1], F32, kind="ExternalInput")
    be1 = nc.dram_tensor("be1", [1, H1], F32, kind="ExternalInput")
    mw2 = nc.dram_tensor("mw2", [H1, H2], BF16, kind="ExternalInput")
    b2 = nc.dram_tensor("b2", [1, H2], F32, kind="ExternalInput")
    g2 = nc.dram_tensor("g2", [1, H2], F32, kind="ExternalInput")
    be2 = nc.dram_tensor("be2", [1, H2], F32, kind="ExternalInput")
    mw3 = nc.dram_tensor("mw3", [H2, NCLS], BF16, kind="ExternalInput")
    b3 = nc.dram_tensor("b3", [1, NCLS], F32, kind="ExternalInput")
    out_d = nc.dram_tensor("out", [n_graphs, NCLS], F32, kind="ExternalOutput")

    with tile.TileContext(nc) as tc:
        with tc.tile_pool(name="consts", bufs=1) as consts:
            w0t = consts.tile([128, NCH, D], BF16)
            for mc in range(NCH):
                nc.sync.dma_start(
                    out=w0t[0 : CHS[mc], mc, :],
                    in_=w0[128 * mc : 128 * mc + CHS[mc], :],
                )
            w1ct = consts.tile([128, D], BF16)
            nc.sync.dma_start(out=w1ct[:, :], in_=w1c[:, :])
            w2ct = consts.tile([128, D], BF16)
            nc.sync.dma_start(out=w2ct[:, :], in_=w2c[:, :])
            t0t = consts.tile([128, 1], F32)
            nc.sync.dma_start(out=t0t[:, :], in_=t0[:, :])
            t1t = consts.tile([128, 1], F32)
            nc.sync.dma_start(out=t1t[:, :], in_=t1[:, :])
            t2t = consts.tile([128, 1], F32)
            nc.sync.dma_start(out=t2t[:, :], in_=t2[:, :])
            ident = consts.tile([128, 128], BF16)
            make_identity(nc, ident[:, :])
            ones = consts.tile([128, 1], BF16)
            nc.vector.memset(ones[:, :], 1.0)
            w2t = consts.tile([128, 2, H2], BF16)
            for c in range(2):
                nc.sync.dma_start(
                    out=w2t[:, c, :], in_=mw2[128 * c : 128 * (c + 1), :]
                )
            w3t = consts.tile([H2, NCLS], BF16)
            nc.sync.dma_start(out=w3t[:, :], in_=mw3[:, :])
            b1b = consts.tile([BG, H1], F32)
            nc.sync.dma_start(out=b1b[:, :], in_=b1[0:1, :].to_broadcast([BG, H1]))
            g1b = consts.tile([BG, H1], F32)
            nc.sync.dma_start(out=g1b[:, :], in_=g1[0:1, :].to_broadcast([BG, H1]))
            be1b = consts.tile([BG, H1], F32)
            nc.sync.dma_start(out=be1b[:, :], in_=be1[0:1, :].to_broadcast([BG, H1]))
            b2b = consts.tile([BG, H2], F32)
            nc.sync.dma_start(out=b2b[:, :], in_=b2[0:1, :].to_broadcast([BG, H2]))
            g2b = consts.tile([BG, H2], F32)
            nc.sync.dma_start(out=g2b[:, :], in_=g2[0:1, :].to_broadcast([BG, H2]))
            be2b = consts.tile([BG, H2], F32)
            nc.sync.dma_start(out=be2b[:, :], in_=be2[0:1, :].to_broadcast([BG, H2]))
            b3b = consts.tile([BG, NCLS], F32)
            nc.sync.dma_start(out=b3b[:, :], in_=b3[0:1, :].to_broadcast([BG, NCLS]))
            epst = consts.tile([BG, 1], F32)
            nc.vector.memset(epst[:, :], EPS_LN)
            x3a = consts.tile([128, n_graphs * N], BF16)

            # ---------------- Phase A: per-graph GCN ----------------
            with (
                tc.tile_pool(name="sctp", bufs=2) as sctp,
                tc.tile_pool(name="up", bufs=2) as up,
                tc.tile_pool(name="xtp", bufs=3) as xtp,
                tc.tile_pool(name="dvp", bufs=2) as dvp,
                tc.tile_pool(name="drd", bufs=2, space="DRAM") as drd,
                tc.tile_pool(name="zp", bufs=4, space="PSUM") as zp,
                tc.tile_pool(name="vp", bufs=2, space="PSUM") as vp,
                tc.tile_pool(name="dp", bufs=2, space="PSUM") as dp,
            ):
                for g in range(n_graphs):
                    sct_t = [
                        sctp.tile([128, N], BF16, tag=f"sct{mc}", name=f"sct_t{mc}") for mc in range(NCH)
                    ]
                    for mc in range(NCH):
                        nc.sync.dma_start(
                            out=sct_t[mc][0 : CHS[mc], :],
                            in_=sct[g, 128 * mc : 128 * mc + CHS[mc], :],
                        )

                    # z1 = sc @ W0' (node-major chunks) + degree row-sums
                    degp = dp.tile([128, NCH], F32, tag="deg", name="degp")
                    zb = [zp.tile([128, D], F32, tag="z", name=f"zb{_c}") for _c in range(NCH)]
                    for mc in range(NCH):
                        szm = CHS[mc]
                        for ncc in range(NCH):
                            szn = CHS[ncc]
                            lhsT = sct_t[mc][0:szm, 128 * ncc : 128 * ncc + szn]
                            nc.tensor.matmul(
                                zb[ncc][0:szn, :],
                                lhsT,
                                w0t[0:szm, mc, :],
                                start=(mc == 0),
                                stop=(mc == NCH - 1),
                            )
                            nc.tensor.matmul(
                                degp[0:szn, ncc : ncc + 1],
                                lhsT,
                                ones[0:szm, :],
                                start=(mc == 0 and ncc == 0),
                                stop=(mc == NCH - 1 and ncc == NCH - 1),
                                skip_group_check=True,
                            )

                    # dinv = 1/sqrt(deg + 1)  (node-major [128, NCH])
                    sq = dvp.tile([128, NCH], F32, tag="sq", name="sq")
                    dinvf = dvp.tile([128, NCH], F32, tag="dinvf", name="dinvf")
                    dinvb = dvp.tile([128, NCH], BF16, tag="dinvb", name="dinvb")
                    for psl, csl in (((0, 128), (0, NCH - 1)), ((0, CHS[-1]), (NCH - 1, NCH))):
                        nc.scalar.activation(
                            sq[psl[0] : psl[1], csl[0] : csl[1]],
                            degp[psl[0] : psl[1], csl[0] : csl[1]],
                            AF.Sqrt,
                            bias=1.0,
                        )
                        nc.vector.reciprocal(
                            dinvf[psl[0] : psl[1], csl[0] : csl[1]],
                            sq[psl[0] : psl[1], csl[0] : csl[1]],
                        )
                        nc.vector.tensor_copy(
                            dinvb[psl[0] : psl[1], csl[0] : csl[1]],
                            dinvf[psl[0] : psl[1], csl[0] : csl[1]],
                        )
                    # row-form + broadcast of dinv (for the column scaling)
                    drow = drd.tile([1, N], BF16, tag="drow", name="drow")
                    for mc in range(NCH):
                        nc.sync.dma_start(
                            out=drow[0:1, 128 * mc : 128 * mc + CHS[mc]],
                            in_=dinvb[0 : CHS[mc], mc : mc + 1],
                        )
                    dbc = dvp.tile([128, N], BF16, tag="dbc", name="dbc")
                    nc.sync.dma_start(out=dbc[:, :], in_=drow[0:1, :].to_broadcast([128, N]))

                    # normalize sct in place -> A_hatT = (scT + I) * dinv[m] * dinv[n]
                    for mc in range(NCH):
                        szm = CHS[mc]
                        dblk = sct_t[mc][0:szm, 128 * mc : 128 * mc + szm]
                        nc.vector.tensor_add(dblk, dblk, ident[0:szm, 0:szm])
                        nc.vector.tensor_scalar_mul(
                            sct_t[mc][0:szm, :],
                            sct_t[mc][0:szm, :],
                            dinvf[0:szm, mc : mc + 1],
                        )
                        nc.vector.tensor_mul(
                            sct_t[mc][0:szm, :], sct_t[mc][0:szm, :], dbc[0:szm, :]
                        )

                    tbias = (t0t, t1t, t2t)
                    for layer in range(3):
                        if layer > 0:
                            w_t = w1ct if layer == 1 else w2ct
                            zb = [zp.tile([128, D], F32, tag="z", name=f"zb{_c}") for _c in range(NCH)]
                            for ncc in range(NCH):
                                szn = CHS[ncc]
                                nc.tensor.matmul(
                                    zb[ncc][0:szn, :],
                                    xt[:, 128 * ncc : 128 * ncc + szn],
                                    w_t[:, :],
                                    start=True,
                                    stop=True,
                                )
                        # evict z chunks (cast to bf16), split across ACT/DVE
                        u_t = [
                            up.tile([128, D], BF16, tag=f"u{c}", name=f"u_t{c}") for c in range(NCH)
                        ]
                        for ncc in range(NCH):
                            szn = CHS[ncc]
                            if ncc % 2 == 0:
                                nc.scalar.copy(u_t[ncc][0:szn, :], zb[ncc][0:szn, :])
                            else:
                                nc.vector.tensor_copy(
                                    u_t[ncc][0:szn, :], zb[ncc][0:szn, :]
                                )
                        # aggregate: v^T = sum_m u[m, :]^T A_hatT[m, :]
                        vt = vp.tile([128, N], F32, tag="v", name="vt")
                        for mc in range(NCH):
                            szm = CHS[mc]
                            nc.tensor.matmul(
                                vt[:, :],
                                u_t[mc][0:szm, :],
                                sct_t[mc][0:szm, :],
                                start=(mc == 0),
                                stop=(mc == NCH - 1),
                            )
                        if layer < 2:
                            xt = xtp.tile([128, N], BF16, tag="xt", name="xt")
                            nc.scalar.activation(
                                xt[:, :], vt[:, :], AF.Relu, bias=tbias[layer][:, 0:1]
                            )
                        else:
                            nc.scalar.activation(
                                x3a[:, g * N : (g + 1) * N],
                                vt[:, :],
                                AF.Identity,
                                bias=tbias[layer][:, 0:1],
                            )

            # ---------------- Phase B: batched MLP head ----------------
            with (
                tc.tile_pool(name="wtp", bufs=8) as wtp,
                tc.tile_pool(name="mls", bufs=2) as mls,
                tc.tile_pool(name="po1", bufs=1, space="PSUM") as po1,
                tc.tile_pool(name="ptp", bufs=2, space="PSUM") as ptp,
                tc.tile_pool(name="po23", bufs=2, space="PSUM") as po23,
            ):
                ng = n_graphs
                x3r = x3a[:, :].rearrange("p (g n) -> p n g", n=N)
                o1 = po1.tile([ng, H1], F32, tag="o1", name="o1")
                for n in range(N):
                    wt = wtp.tile([128, H1], BF16, tag="w1", name="wt")
                    nc.sync.dma_start(
                        out=wt[:, :], in_=mw1[128 * n : 128 * (n + 1), :]
                    )
                    nc.tensor.matmul(
                        o1[:, :],
                        x3r[:, n, :],
                        wt[:, :],
                        start=(n == 0),
                        stop=(n == N - 1),
                    )

                def layernorm_relu(src_psum, bb, gb, beb, width, out_dt):
                    hs = mls.tile([ng, width], F32, tag=f"hs{width}", name=f"hs{width}")
                    nc.vector.tensor_add(hs[:, :], src_psum[:, :], bb[0:ng, :])
                    stats = mls.tile([ng, 6], F32, tag="stats", name="stats")
                    nc.vector.bn_stats(stats[:, :], hs[:, :])
                    mv = mls.tile([ng, 2], F32, tag="mv", name="mv")
                    nc.vector.bn_aggr(mv[:, :], stats[:, :])
                    sq1 = mls.tile([ng, 1], F32, tag="sq1", name="sq1")
                    nc.scalar.activation(
                        sq1[:, :], mv[:, 1:2], AF.Sqrt, bias=epst[0:ng, 0:1]
                    )
                    r1 = mls.tile([ng, 1], F32, tag="r1", name="r1")
                    nc.vector.reciprocal(r1[:, :], sq1[:, :])
                    yn = mls.tile([ng, width], F32, tag=f"yn{width}", name=f"yn{width}")
                    nc.vector.tensor_scalar(
                        yn[:, :], hs[:, :], mv[:, 0:1], r1[:, 0:1],
                        op0=OP.subtract, op1=OP.mult,
                    )
                    nc.vector.tensor_mul(yn[:, :], yn[:, :], gb[0:ng, :])
                    nc.vector.tensor_add(yn[:, :], yn[:, :], beb[0:ng, :])
                    y = mls.tile([ng, width], out_dt, tag=f"y{width}", name=f"y{width}")
                    nc.scalar.activation(y[:, :], yn[:, :], AF.Relu)
                    return y

                y1 = layernorm_relu(o1, b1b, g1b, be1b, H1, BF16)
                y1T = []
                for c in range(2):
                    tp = ptp.tile([128, ng], BF16, tag="tp", name="tp")
                    nc.tensor.transpose(
                        tp[:, :], y1[0:ng, 128 * c : 128 * (c + 1)], ident[0:ng, 0:ng]
                    )
                    y1Ts = mls.tile([128, ng], BF16, tag=f"y1T{c}", name=f"y1Ts{c}")
                    nc.scalar.copy(y1Ts[:, :], tp[:, :])
                    y1T.append(y1Ts)
                o2 = po23.tile([ng, H2], F32, tag="o2", name="o2")
                for c in range(2):
                    nc.tensor.matmul(
                        o2[:, :], y1T[c][:, 0:ng], w2t[:, c, :],
                        start=(c == 0), stop=(c == 1),
                    )
                y2 = layernorm_relu(o2, b2b, g2b, be2b, H2, BF16)
                tp3 = ptp.tile([128, ng], BF16, tag="tp", name="tp")
                nc.tensor.transpose(tp3[0:H2, :], y2[0:ng, :], ident[0:ng, 0:ng])
                y2Ts = mls.tile([H2, ng], BF16, tag="y2T", name="y2Ts")
                nc.scalar.copy(y2Ts[:, :], tp3[0:H2, :])
                o3 = po23.tile([ng, NCLS], F32, tag="o3", name="o3")
                nc.tensor.matmul(
                    o3[:, :], y2Ts[:, 0:ng], w3t[:, :], start=True, stop=True
                )
                outs = mls.tile([ng, NCLS], F32, tag="outs", name="outs")
                nc.vector.tensor_add(outs[:, :], o3[:, :], b3b[0:ng, :])
                nc.sync.dma_start(out=out_d[:, :], in_=outs[:, :])
    return nc


_NC_CACHE = {}


def _get_nc(n_graphs=BG):
    if n_graphs not in _NC_CACHE:
        nc = bacc.Bacc("TRN2", target_bir_lowering=False, debug=False)
        _emit(nc, n_graphs)
        nc.compile()
        _NC_CACHE[n_graphs] = nc
    return _NC_CACHE[n_graphs]


def _fold_params(params):
    """Fold BN (eval mode) into conv weights/biases; cast for device."""
    ins = {}
    convs_w = []
    convs_t = []
    for j in range(3):
        s = np.asarray(params["bn_gamma"][j], np.float32) / np.sqrt(
            np.asarray(params["bn_var"][j], np.float32) + EPS_BN
        )
        t = (
            np.asarray(params["conv_b"][j], np.float32)
            - np.asarray(params["bn_mean"][j], np.float32)
        ) * s + np.asarray(params["bn_beta"][j], np.float32)
        w = np.asarray(params["conv_w"][j], np.float32) * s[None, :]
        convs_w.append(np.ascontiguousarray(w).astype(NPBF16))
        convs_t.append(np.ascontiguousarray(t.reshape(D, 1), np.float32))
    ins["w0"], ins["w1c"], ins["w2c"] = convs_w
    ins["t0"], ins["t1"], ins["t2"] = convs_t
    ins["mw1"] = np.ascontiguousarray(np.asarray(params["w1"], np.float32)).astype(NPBF16)
    ins["b1"] = np.asarray(params["b1"], np.float32).reshape(1, H1).copy()
    ins["g1"] = np.asarray(params["ln1_g"], np.float32).reshape(1, H1).copy()
    ins["be1"] = np.asarray(params["ln1_b"], np.float32).reshape(1, H1).copy()
    ins["mw2"] = np.ascontiguousarray(np.asarray(params["w2"], np.float32)).astype(NPBF16)
    ins["b2"] = np.asarray(params["b2"], np.float32).reshape(1, H2).copy()
    ins["g2"] = np.asarray(params["ln2_g"], np.float32).reshape(1, H2).copy()
    ins["be2"] = np.asarray(params["ln2_b"], np.float32).reshape(1, H2).copy()
    ins["mw3"] = np.ascontiguousarray(np.asarray(params["w3"], np.float32)).astype(NPBF16)
    ins["b3"] = np.asarray(params["b3"], np.float32).reshape(1, NCLS).copy()
    return ins


def _prep_in_maps(sc_matrix, params, n_graphs=BG, n_cores=NCORES):
    base = _fold_params(params)
    sc = np.asarray(sc_matrix, np.float32)
    in_maps = []
    for c in range(n_cores):
        shard = sc[c * n_graphs : (c + 1) * n_graphs]
        sct = np.ascontiguousarray(shard.transpose(0, 2, 1)).astype(NPBF16)
        m = dict(base)
        m["sct"] = sct
        in_maps.append(m)
    return in_maps


def _run(sc_matrix, params, trace=False, tmpdir=None):
    nc = _get_nc(BG)
    in_maps = _prep_in_maps(sc_matrix, params)
    res = run_bass_kernel_spmd(
        nc, in_maps, core_ids=list(range(NCORES)), trace=trace, tmpdir=tmpdir
    )
    out = np.concatenate(
        [np.asarray(res.results[c]["out"], np.float32) for c in range(NCORES)], axis=0
    )
    return out, res


def kernel(fc_matrix=None, sc_matrix=None, params=None):
    out, _ = _run(sc_matrix, params, trace=False)
    return out
